# revision 23
# baseline (speedup 1.0000x reference)
"""Depthwise 3x3 CNN combo kernel for TRN2 (8 NeuronCores, channel-parallel).

Computes  out = relu(x*a0 + dwconv(x,w1)*a1 + dwconv(x,w2)*a2 + dwconv(x,w3)*a3)
for x [8, 256, 128, 128] f32 by folding everything into a single 9-tap
depthwise conv (conv is linear in the weights; the residual a0*x is the
center tap):  w_eff = a1*w1 + a2*w2 + a3*w3,  w_eff[:,1,1] += a0.

Sharding: CHANNELS across the 8 cores (32 channels x 8 batch images per
core).  Per-core layout puts image ROWS on the partitions:

  x tile  [y=128, (c, b, w=130)]   (w padded 1 left / 1 right with zeros,
                                    host-prepadded so DMA runs are 2080B)

The vertical 3-tap conv then becomes a matmul over the partition (row)
dim with a TRIDIAGONAL stationary matrix T_dx[yi, yo] = w_eff[c, yi-yo+1, dx]
(one matrix per channel and horizontal offset dx).  Each streamed rhs
column picks up all 3 vertical taps at once, so the full 9-tap conv needs
only 3 accumulating matmuls per psum tile:

  psum[yo, (b, t)] += sum_yi T_dx[yi, yo] * xt[yi, (b, t + dx)]   dx = 0..2

with the dx shift handled as a free-dim offset into the padded row.

The kernel is HBM-bandwidth bound (~415 GB/s/core sustained, measured):
x in 8.5MB + W + y out 8.4MB.  To cut W traffic in half, the tridiagonal
matrices are stored in HBM as INT8 (band = round(w_eff * 128), exact in
bf16 after a *2^-7 rescale since |values| <= 127) and cast to bf16
on-chip by DVE (one tensor_scalar multiply per 4-channel chunk).  The
weight quantization error is ~2^-8 ABSOLUTE per tap, giving rel-err
~9.4e-3 (vs 4.6e-3 all-bf16) -- well under the 2e-2 gate.  If w_eff ever
exceeded int8 range, the host halves the weight scale and doubles x by
the same power of two (exact bf16 exponent shifts), so the baked 1/128
immediate stays correct.

DMA plumbing (measured): sync (SP) and scalar (Activation) each drive a
HWDGE queue; gpsimd drives a SWDGE queue; the three together saturate
the ~415 GB/s/core HBM share, so queue balance only needs to be rough:
x alternates sync/scalar (2-ch chunks, 2080B runs), W int8 alternates
(4-ch chunks), y rides mostly gpsimd (pairs 2-11) with head/tail pairs
on the HW queues, and the final pair is split into two single-channel
stores issued in parallel on sync+scalar to shorten the drain.  PSUM
accumulates in f32; relu+downcast to bf16 alternates DVE and ScalarE
(one contiguous 2048-elem op per channel); host upcasts.
"""

import numpy as np

import concourse.bacc as bacc
import concourse.mybir as mybir
from concourse import bass_utils
from concourse.tile import TileContext

# Problem constants (hardcoded per contract).
B = 8
C = 256
H = 128
W = 128
NCORES = 8

CPC = C // NCORES   # channels per core
P = 128             # partitions (= H rows)
WP = W + 2          # padded row width (1 zero col each side)

F32 = mybir.dt.float32
BF16 = mybir.dt.bfloat16
I8 = mybir.dt.int8

WSCALE = 128.0      # weight quantization scale (int8 band = w_eff * 128)


def build_tile_kernel(tc, y_ap, x_ap, w_ap):
    nc = tc.nc
    relu = mybir.ActivationFunctionType.Relu

    with (
        tc.tile_pool(name="xspool", bufs=4) as xspool,
        tc.tile_pool(name="xpool", bufs=14) as xpool,
        tc.tile_pool(name="w8pool", bufs=8) as w8pool,
        tc.tile_pool(name="wbpool", bufs=4) as wbpool,
        tc.tile_pool(name="warmp", bufs=1) as warmp,
        tc.tile_pool(name="qwp", bufs=2) as qwp,
        tc.tile_pool(name="psum", bufs=4, space="PSUM") as psum_pool,
        # Deep output pool: a y store can sit several microseconds behind
        # loads in its DMA queue; with few bufs the relu of pair p
        # WAR-stalls on the store of pair p-bufs, which stalls psum reuse
        # and ultimately the PE (measured 2-3us dents in the MM stream).
        tc.tile_pool(name="opool", bufs=12) as opool,
    ):
        # y pairs 0..9 ride the gpsimd SWDGE queue (it carries no loads,
        # so mid-kernel stores are never stuck behind x/w DMAs, and its
        # laggy completion semaphores -- ~6us after the data, measured --
        # stay off the final barrier's critical path).  Pairs 10..11 ride
        # the HW queues whole; tail pairs 12..15 are split into
        # single-channel stores on both HW queues, whose loads have
        # drained by then.
        GP_PAIRS = set(range(0, 10))

        # x chunks: channels 0-3 as singles (the first loads gate the MM
        # stream start; a 266KB single lands ~1.2us earlier than a pair),
        # then 2-channel chunks.
        XCH = [(c, 1) for c in range(4)] + [(4 + 2 * i, 2) for i in range(14)]
        chan_chunk = {}
        for idx, (c0, wd) in enumerate(XCH):
            for cc in range(c0, c0 + wd):
                chan_chunk[cc] = (idx, cc - c0)

        xts = [None] * len(XCH)
        w8s = [None] * 8   # 4-channel int8 W chunks, c = 4j
        wbs = [None] * 8   # bf16-cast W chunks

        def emit_xchunk(idx):
            c0, wd = XCH[idx]
            pool, tg = (xspool, "xs") if wd == 1 else (xpool, "xt")
            xt = pool.tile([P, wd, B, WP], BF16, name="xt", tag=tg)
            q = nc.sync if idx % 2 == 0 else nc.scalar
            q.dma_start(xt[:], x_ap[:, c0 : c0 + wd])
            xts[idx] = xt

        def emit_w(j):
            c0 = 4 * j
            wc = w8pool.tile([P, 4, 3, P], I8, name="w8", tag="w8")
            # w0 rides sync: the scalar HWDGE queue spins up ~2us later
            # (profiled), and w0 gates the first cast and hence MM #0.
            q = nc.sync if j % 2 == 0 else nc.scalar
            q.dma_start(wc[:], w_ap[:, c0 : c0 + 4])
            w8s[j] = wc

        def emit_cast(j):
            # DVE (gpsimd's tensor ops are ~20x slower -- measured 22us
            # per chunk).  The W DMA trigger runs >=8 channels ahead of
            # the cast, so the cast never blocks DVE's strict-FIFO queue
            # (a blocked cast would stall the relus behind it and,
            # through psum-tile reuse, the PE).
            wb = wbpool.tile([P, 4, 3, P], BF16, name="wb", tag="wb")
            src = w8s[j][:].rearrange("p c d k -> p (c d k)")
            dst = wb[:].rearrange("p c d k -> p (c d k)")
            nc.vector.tensor_scalar(
                dst, src, 1.0 / WSCALE, None, op0=mybir.AluOpType.mult
            )
            wbs[j] = wb

        # Four explicit psum tiles rotated by channel: with per-channel
        # pool allocation the scheduler reused a freed bank after only 2
        # channels, coupling the PE to the relu engines at 2-channel
        # slack (measured 2us stalls).  Explicit tiles give 4-channel
        # WAR slack.
        ps_tiles = [
            psum_pool.tile([P, 2, 512], F32, name=f"ps{i}", tag="ps")
            for i in range(4)
        ]

        # PE warm-up: 12 dummy matmuls on a zeroed tile, issued during
        # the DMA fill so the HAM clock gate reaches K=8/8 (2.4 GHz)
        # before the first real matmul (otherwise the first ~8 MMs run at
        # 1.2 GHz, ~1.7us lost).  They write ps_tiles[0], which channel
        # 0's accumulation group later overwrites with start=True.  The
        # memset rides ScalarE, whose program starts ~1.3us before DVE's.
        # Queue prewarm: the first DMA on each HWDGE queue pays a 1.5-2us
        # ring spin-up (measured trigger->first-packet).  A throwaway 64B
        # DMA issued as each engine's first instruction absorbs that
        # latency before the real w0/x0 loads arrive.
        for qeng in (nc.sync, nc.scalar):
            scr = qwp.tile([1, 64], I8, name="qw", tag="qw")
            qeng.dma_start(scr[:], w_ap[0:1, 0:1, 0:1, 0:64])

        warm = warmp.tile([P, 640], BF16, name="warm", tag="warm")
        nc.vector.memset(warm[:], 0.0)
        for _ in range(10):
            nc.tensor.matmul(
                ps_tiles[0][:, 0, :], lhsT=warm[:, 0:128],
                rhs=warm[:, 128:640],
                start=True, stop=True, skip_group_check=True,
            )

        # Prefetch: W early on each queue (late W blocks a cast at the
        # head of DVE's strict FIFO, which stalls the relus queued behind
        # it -- measured as a 6us regression when W trailed the x
        # singles), interleaved with the x singles in need order.
        emit_w(0); emit_xchunk(0); emit_xchunk(1); emit_w(1)
        emit_xchunk(2); emit_xchunk(3); emit_w(2); emit_w(3)
        emit_xchunk(4); emit_xchunk(5)
        emit_cast(0); emit_cast(1)
        next_chunk = 6

        ot = None
        for c in range(CPC):
            if c % 2 == 0:
                # keep x prefetch ~9 channels ahead
                while next_chunk < len(XCH) and XCH[next_chunk][0] <= c + 9:
                    emit_xchunk(next_chunk)
                    next_chunk += 1
            if c % 4 == 0:
                j = c // 4 + 4
                if j < 8:
                    emit_w(j)
                j = c // 4 + 2
                if 2 <= j < 8:
                    emit_cast(j)
            if c % 2 == 0:
                # 2-channel output tile -> one 4KB-run DMA per 2 channels.
                ot = opool.tile([P, 2, B, W], BF16, name="ot", tag="ot")
            ki, kp = chan_chunk[c]
            ps = ps_tiles[c % 4]
            if c >= 30:
                # Terminal pair: each 4-image half accumulates in its OWN
                # psum tile (all four are free by now), so the half relus
                # and ~131KB half stores overlap the remaining matmuls
                # with no tile-granular WAR coupling, and the chain after
                # the very last matmul is only relu(half)+store(half).
                for h in range(2):
                    pst = ps_tiles[(c % 4) - 2 * h]  # 30:t2,t0  31:t3,t1
                    for dx in range(3):
                        nc.tensor.matmul(
                            pst[:, 0, :],
                            lhsT=wbs[c // 4][:, c % 4, dx, :],
                            rhs=xts[ki][:, kp, 4 * h : 4 * h + 4,
                                        dx : dx + W],
                            start=(dx == 0),
                            stop=(dx == 2),
                            skip_group_check=True,
                        )
                    src = pst[:, 0, :]
                    dst = ot[:, c % 2, 4 * h : 4 * h + 4].rearrange(
                        "p b w -> p (b w)")
                    if h == 0:
                        nc.vector.tensor_scalar_max(dst, src, 0.0)
                    else:
                        nc.scalar.activation(dst, src, relu)
                    q = nc.sync if h == 0 else nc.scalar
                    q.dma_start(
                        y_ap[:, c : c + 1, 4 * h : 4 * h + 4],
                        ot[:, c % 2 : c % 2 + 1, 4 * h : 4 * h + 4],
                    )
                continue
            for dx in range(3):
                lhsT = wbs[c // 4][:, c % 4, dx, :]
                for g in range(2):
                    nc.tensor.matmul(
                        ps[:, g, :],
                        lhsT=lhsT,
                        rhs=xts[ki][:, kp, 4 * g : 4 * g + 4, dx : dx + W],
                        start=(dx == 0),
                        stop=(dx == 2),
                        skip_group_check=True,
                    )
            # relu + bf16 downcast, alternating DVE / ScalarE.  (GpSimd
            # cannot read PSUM.)
            src = ps[:].rearrange("p g w -> p (g w)")
            dst = ot[:, c % 2].rearrange("p b w -> p (b w)")
            if c % 2 == 0:
                nc.vector.tensor_scalar_max(dst, src, 0.0)
            else:
                nc.scalar.activation(dst, src, relu)
            if c % 2 == 1:
                pair = c // 2
                if pair >= 12:
                    # Split the tail pairs: two single-channel stores in
                    # parallel on both HW queues to shorten the drain.
                    nc.sync.dma_start(y_ap[:, c - 1 : c], ot[:, 0:1])
                    nc.scalar.dma_start(y_ap[:, c : c + 1], ot[:, 1:2])
                elif pair in GP_PAIRS:
                    nc.gpsimd.dma_start(y_ap[:, c - 1 : c + 1], ot[:])
                else:
                    q = nc.sync if pair % 2 == 0 else nc.scalar
                    q.dma_start(y_ap[:, c - 1 : c + 1], ot[:])


def host_weights(a, w1, w2, w3):
    """Fold the 4-way combine into one 9-tap depthwise kernel w_eff."""
    a = np.asarray(a, np.float64)
    w_eff = (
        a[1] * np.asarray(w1, np.float64)[:, 0]
        + a[2] * np.asarray(w2, np.float64)[:, 0]
        + a[3] * np.asarray(w3, np.float64)[:, 0]
    )  # [C, 3, 3]
    w_eff[:, 1, 1] += a[0]
    return w_eff


def host_quant(w_eff):
    """Quantize w_eff to int8 at the largest power-of-2 scale <= 128 that
    fits; return (band int8 [C,3,3], x power-of-2 compensation factor).

    The kernel bakes a 1/128 rescale, so if the weight scale had to drop
    to 128/2^m, x is multiplied by 2^m on the host (exact in bf16)."""
    ws = WSCALE
    xcomp = 1.0
    while np.abs(np.rint(w_eff * ws)).max() > 127:
        ws /= 2.0
        xcomp *= 2.0
    wq = np.rint(w_eff * ws).astype(np.int8)
    return wq, xcomp


def host_tridiag_i8(wq):
    """[yi, c, dx, yo] int8 tridiag stationary matrices:
    T[yi,c,dx,yo] = wq[c, yi-yo+1, dx] for |yi-yo| <= 1."""
    T = np.zeros((P, C, 3, P), np.int8)
    for dy in range(3):
        yo = np.arange(max(0, 1 - dy), min(P, P + 1 - dy))
        yi = yo + dy - 1
        T[yi, :, :, yo] = wq[:, dy, :]
    return T


def host_inputs(x, xcomp):
    """[y, c, b, w+2] zero-padded bf16, split per core along c."""
    import ml_dtypes

    xb = np.asarray(x).astype(ml_dtypes.bfloat16)  # [b, c, y, w]
    if xcomp != 1.0:
        xb = (xb * ml_dtypes.bfloat16(xcomp)).astype(ml_dtypes.bfloat16)
    X = np.zeros((P, C, B, WP), ml_dtypes.bfloat16)
    X[:, :, :, 1 : W + 1] = xb.transpose(2, 1, 0, 3)
    return X


_PROGRAM = None


def _get_program():
    global _PROGRAM
    if _PROGRAM is None:
        nc = bacc.Bacc(
            "TRN2", target_bir_lowering=False, debug=False,
            enable_partition_id=False,
        )
        x_t = nc.dram_tensor("x", [P, CPC, B, WP], BF16, kind="ExternalInput")
        y_t = nc.dram_tensor("y", [P, CPC, B, W], BF16, kind="ExternalOutput")
        w_t = nc.dram_tensor("w", [P, CPC, 3, P], I8, kind="ExternalInput")
        with TileContext(nc) as tc:
            build_tile_kernel(tc, y_t.ap(), x_t.ap(), w_t.ap())
        nc.compile()
        _PROGRAM = nc
    return _PROGRAM


def kernel(x, a, w1, w2, w3, _trace=False, _trace_kwargs=None):
    w_eff = host_weights(a, w1, w2, w3)
    wq, xcomp = host_quant(w_eff)
    X = host_inputs(x, xcomp)
    T = host_tridiag_i8(wq)
    in_maps = []
    for i in range(NCORES):
        cs = slice(CPC * i, CPC * (i + 1))
        in_maps.append({
            "x": np.ascontiguousarray(X[:, cs]),
            "w": np.ascontiguousarray(T[:, cs]),
        })
    nc = _get_program()
    res = bass_utils.run_bass_kernel_spmd(
        nc, in_maps, core_ids=list(range(NCORES)), trace=_trace,
        **(_trace_kwargs or {}),
    )
    # res y: [yi, cc, b, w] per core -> out[b, core*CPC+cc, y, w]
    out = np.stack(
        [np.asarray(r["y"], np.float32) for r in res.results], axis=0
    )
    out = out.transpose(3, 0, 2, 1, 4).reshape(B, C, H, W)
    if _trace:
        return out, res
    return out
